# revision 6
# baseline (speedup 1.0000x reference)
"""Multi-head causal attention (b=4, n=2048, d_model=1024, 16 heads) on 8
Trainium2 NeuronCores.

Sharding: core c = (batch b = c//2, head-group hg = c%2); each core computes
one batch with 8 heads (tensor-parallel split of w_q/w_k/w_v by rows and w_o
by columns) and returns a partial [2048, 1024] output; host sums the two
head-group partials per batch.

Per-core device algorithm (all matmuls fp32r = 1 PE cycle/column):
  Phase 1: qT/kT = (X @ W.T).T via PE with host-transposed inputs; v in
           natural [seq, d] layout with an appended ones column (gives
           softmax denominators for free in the PV matmul).
  Phase 2: per q-tile t (512 q) and head-pair g: scores S^T[k,q] blocks via
           2-way row-tiled matmuls (dk=64 each), exp on ACT (scale=1/8,
           no max subtraction: |s|/8 < ~3), causal mask multiply on diagonal
           blocks, PV accumulation into [65, 512] PSUM (row 64 = rowsum).
           Stage O^T + rowsums to SBUF, reciprocal, PE-broadcast, rescale.
  Phase 3: O-projection out[seq, 1024] = O^T.T @ w_o_slice.T per q-tile.
"""

import numpy as np

B = 4
N = 2048
D_MODEL = 1024
DK = 64
NT = 4          # q tiles of 512
QT = 512        # q tile size
KB = 128        # key block size
N_CORES = 8

_CACHE = {}


def _round_f32r(x: np.ndarray) -> np.ndarray:
    """Round fp32 to fp32r (11 explicit mantissa bits, RNE) host-side."""
    u = np.ascontiguousarray(x, dtype=np.float32).view(np.uint32)
    r = (u + np.uint32(0x7FF) + ((u >> np.uint32(12)) & np.uint32(1))) & np.uint32(
        0xFFFFF000
    )
    return r.view(np.float32)


def _split_sync_waits(nc, max_waits=1):
    """walrus on this image allows only 1 sync-wait command per instruction;
    hoist excess waits onto same-engine NoOps inserted just before."""
    import concourse.mybir as mybir

    n_split = 0
    for fn in nc.m.functions:
        for blk in fn.blocks:
            insts = list(blk.instructions)
            out = []
            for inst in insts:
                si = inst.sync_info
                if si is not None and len(si.on_wait) > max_waits:
                    waits = list(si.on_wait)
                    head, rest = waits[:-max_waits], waits[-max_waits:]
                    while head:
                        chunk, head = head[:max_waits], head[max_waits:]
                        nop = mybir.InstNoOp(
                            name=f"{inst.name}-ws{n_split}-{len(out)}",
                            engine=inst.engine,
                            opcode="NoOp",
                            sync_info=mybir.SyncInfo(on_wait=chunk, on_update=[]),
                            bass_nofuse=True,
                        )
                        out.append(nop)
                    si.on_wait = rest
                    n_split += 1
                out.append(inst)
            if len(out) != len(insts):
                blk.instructions = out
    return n_split


def build_nc():
    import concourse.bass as bass
    import concourse.mybir as mybir
    import concourse.tile as tile
    from concourse.bass import ts

    F32 = mybir.dt.float32
    F32R = mybir.dt.float32r
    AF = mybir.ActivationFunctionType

    nc = bass.Bass("TRN2", target_bir_lowering=False, debug=False)

    qT_d = nc.dram_tensor("qT", [D_MODEL, N], F32R, kind="ExternalInput")
    kT_d = nc.dram_tensor("kT", [D_MODEL, N], F32R, kind="ExternalInput")
    vT_d = nc.dram_tensor("vT", [D_MODEL, N], F32R, kind="ExternalInput")
    wqT_d = nc.dram_tensor("wqT", [D_MODEL, 512], F32R, kind="ExternalInput")
    wkT_d = nc.dram_tensor("wkT", [D_MODEL, 512], F32R, kind="ExternalInput")
    wvT_d = nc.dram_tensor("wvT", [D_MODEL, 512], F32R, kind="ExternalInput")
    woT_d = nc.dram_tensor("woT", [512, D_MODEL], F32R, kind="ExternalInput")
    masks_d = nc.dram_tensor("masks", [4, 128, QT], F32R, kind="ExternalInput")
    onescol_d = nc.dram_tensor("onescol", [128, 8], F32R, kind="ExternalInput")
    sel_d = nc.dram_tensor("sel", [8, 4, 128], F32R, kind="ExternalInput")
    out_d = nc.dram_tensor("out", [N, D_MODEL], F32, kind="ExternalOutput")

    with (
        tile.TileContext(nc) as tc,
        nc.allow_low_precision(reason="fp32r matmuls are intentional"),
    ):
        with (
            tc.tile_pool(name="persist", bufs=1) as persist,
            tc.tile_pool(name="pt_pool", bufs=1) as pt_pool,
            tc.tile_pool(name="outp", bufs=1) as outp,
        ):
            # ---- persistent SBUF tensors (whole-kernel lifetime) ----
            qT_all = persist.tile([128, 4, N], F32R)   # [part, m-block, seq]
            kT_all = persist.tile([128, 4, N], F32R)
            v_all = persist.tile([128, 16, 8, 65], F32R)  # [k-part, sb, head, d+1]
            onescol_sb = persist.tile([128, 8], F32R)
            sel_sb = persist.tile([8, 4, 128], F32R)
            nc.sync.dma_start(out=onescol_sb, in_=onescol_d[:, :])
            nc.sync.dma_start(out=sel_sb, in_=sel_d[:, :, :])

            # ================= Phase 1: projections =================
            with (
                tc.tile_pool(name="w1", bufs=1) as w1,
                tc.tile_pool(name="xs", bufs=4) as xs,
                tc.tile_pool(name="pp", bufs=1, space="PSUM") as pp,
            ):
                wq_sb = w1.tile([128, 8, 512], F32R)
                wk_sb = w1.tile([128, 8, 512], F32R)
                wv_sb = w1.tile([128, 8, 512], F32R)
                for kc in range(8):
                    nc.sync.dma_start(out=wq_sb[:, kc, :], in_=wqT_d[ts(kc, 128), :])
                    nc.sync.dma_start(out=wk_sb[:, kc, :], in_=wkT_d[ts(kc, 128), :])
                    nc.sync.dma_start(out=wv_sb[:, kc, :], in_=wvT_d[ts(kc, 128), :])

                # q/k projections: qT_all[:, m, tsl] = (W X^T) block
                for src_d, w_sb, dst in (
                    (qT_d, wq_sb, qT_all),
                    (kT_d, wk_sb, kT_all),
                ):
                    for t in range(NT):
                        pj = [
                            pp.tile(
                                [128, QT], F32, name=f"pj{m}", tag=f"pj{m}", bufs=2
                            )
                            for m in range(4)
                        ]
                        for kc in range(8):
                            x_t = xs.tile([128, QT], F32R, name="x_t", tag="x_t")
                            nc.sync.dma_start(
                                out=x_t, in_=src_d[ts(kc, 128), ts(t, QT)]
                            )
                            for m in range(4):
                                nc.tensor.matmul(
                                    pj[m],
                                    w_sb[:, kc, ts(m, 128)],
                                    x_t[:, :],
                                    start=(kc == 0),
                                    stop=(kc == 7),
                                )
                        for m in range(4):
                            nc.vector.tensor_copy(dst[:, m, ts(t, QT)], pj[m])

                # v projection: natural [seq, d] layout + ones column
                for t in range(NT):
                    pj = [
                        pp.tile([128, QT], F32, name=f"pj{m}", tag=f"pj{m}", bufs=2)
                        for m in range(4)
                    ]
                    for kc in range(8):
                        x_t = xs.tile([128, QT], F32R, name="x_t", tag="x_t")
                        nc.sync.dma_start(out=x_t, in_=vT_d[ts(kc, 128), ts(t, QT)])
                        for m in range(4):
                            nc.tensor.matmul(
                                pj[m],
                                x_t[:, ts(m, 128)],
                                wv_sb[:, kc, :],
                                start=(kc == 0),
                                stop=(kc == 7),
                            )
                    for m in range(4):
                        sb = t * 4 + m
                        nc.vector.tensor_copy(
                            v_all[:, sb, :, 0:64],
                            pj[m][:, :].rearrange("p (h d) -> p h d", h=8),
                        )
                        nc.vector.tensor_copy(v_all[:, sb, :, 64], onescol_sb)

            # ================= Phase 2+3: attention + O-projection =========
            with (
                tc.tile_pool(name="persist2", bufs=1) as persist2,
                tc.tile_pool(name="ps2", bufs=1, space="PSUM") as ps2,
            ):
                ot_sb = [
                    persist2.tile([128, 4, QT], F32R, name=f"ot_sb{t}", tag=f"ot{t}")
                    for t in range(NT)
                ]
                rs_sb = [
                    persist2.tile([8, QT], F32R, name=f"rs_sb{t}", tag=f"rs{t}")
                    for t in range(NT)
                ]
                recip_sb = [
                    persist2.tile([8, QT], F32R, name=f"recip{t}", tag=f"rc{t}")
                    for t in range(NT)
                ]
                masks_sb = persist2.tile([128, 4, QT], F32R)
                wo_sb = persist2.tile([128, 4, D_MODEL], F32R)

                for r in range(4):
                    nc.sync.dma_start(out=masks_sb[:, r, :], in_=masks_d[r, :, :])
                for g in range(4):
                    nc.sync.dma_start(out=wo_sb[:, g, :], in_=woT_d[ts(g, 128), :])

                for t in range(NT):
                    nkb = 4 * t + 4  # causal: key blocks 0 .. 4t+3
                    for g in range(4):
                        ota = ps2.tile([65, QT], F32, name="ota", tag="ota", bufs=1)
                        otb = ps2.tile([65, QT], F32, name="otb", tag="otb", bufs=1)
                        for j in range(nkb):
                            sa = ps2.tile(
                                [128, QT], F32, name="sa", tag="sa", bufs=3
                            )
                            sb_ = ps2.tile(
                                [128, QT], F32, name="sb_", tag="sb_", bufs=2
                            )
                            nc.tensor.matmul(
                                sa,
                                kT_all[0:64, g, ts(j, 128)],
                                qT_all[0:64, g, ts(t, QT)],
                                start=True,
                                stop=True,
                                tile_position=(0, 0),
                            )
                            nc.tensor.matmul(
                                sb_,
                                kT_all[64:128, g, ts(j, 128)],
                                qT_all[64:128, g, ts(t, QT)],
                                start=True,
                                stop=True,
                                tile_position=(64, 0),
                            )
                            pta = pt_pool.tile(
                                [128, QT], F32R, name="pta", tag="pta", bufs=4
                            )
                            ptb = pt_pool.tile(
                                [128, QT], F32R, name="ptb", tag="ptb", bufs=4
                            )
                            r = j - 4 * t
                            if r <= 0:
                                nc.scalar.activation(pta, sa, AF.Exp, scale=0.125)
                                nc.scalar.activation(ptb, sb_, AF.Exp, scale=0.125)
                            else:
                                z = 128 * r
                                nc.scalar.activation(
                                    pta[:, z:], sa[:, z:], AF.Exp, scale=0.125
                                )
                                nc.scalar.activation(
                                    ptb[:, z:], sb_[:, z:], AF.Exp, scale=0.125
                                )
                                nc.vector.tensor_copy(
                                    pta[:, 0:z], masks_sb[:, 3, 0:z]
                                )
                                nc.vector.tensor_copy(
                                    ptb[:, 0:z], masks_sb[:, 3, 0:z]
                                )
                            if r >= 0:
                                nc.vector.tensor_mul(
                                    pta[:, 128 * max(r, 0) :],
                                    pta[:, 128 * max(r, 0) :],
                                    masks_sb[:, r, 128 * max(r, 0) :],
                                )
                                nc.vector.tensor_mul(
                                    ptb[:, 128 * max(r, 0) :],
                                    ptb[:, 128 * max(r, 0) :],
                                    masks_sb[:, r, 128 * max(r, 0) :],
                                )
                            nc.tensor.matmul(
                                ota,
                                v_all[:, j, 2 * g, :],
                                pta[:, :],
                                start=(j == 0),
                                stop=(j == nkb - 1),
                            )
                            nc.tensor.matmul(
                                otb,
                                v_all[:, j, 2 * g + 1, :],
                                ptb[:, :],
                                start=(j == 0),
                                stop=(j == nkb - 1),
                            )
                        # stage O^T and rowsums to SBUF
                        nc.vector.tensor_copy(ot_sb[t][0:64, g, :], ota[0:64, :])
                        nc.vector.tensor_copy(ot_sb[t][64:128, g, :], otb[0:64, :])
                        tmp_rs = pt_pool.tile(
                            [1, 2, QT], F32R, name="tmp_rs", tag="tmp_rs", bufs=2
                        )
                        nc.vector.tensor_copy(tmp_rs[0:1, 0, :], ota[64:65, :])
                        nc.vector.tensor_copy(tmp_rs[0:1, 1, :], otb[64:65, :])
                        nc.sync.dma_start(
                            out=rs_sb[t][2 * g : 2 * g + 2, :], in_=tmp_rs[0:1, :, :]
                        )

                    # normalize this q-tile: recip of gathered rowsums,
                    # PE-broadcast per head pair via K=8 selector matmul
                    nc.vector.reciprocal(recip_sb[t], rs_sb[t])
                    for g in range(4):
                        bc = ps2.tile([128, QT], F32, name="bc", tag="aux", bufs=1)
                        nc.tensor.matmul(
                            bc,
                            sel_sb[:, g, :],
                            recip_sb[t][:, :],
                            start=True,
                            stop=True,
                        )
                        nc.vector.tensor_mul(
                            ot_sb[t][:, g, :], ot_sb[t][:, g, :], bc
                        )

                    # ---- Phase 3 for this q-tile: O-projection ----
                    for mm in range(4):
                        m = 4 * t + mm
                        for n2 in range(2):
                            po = ps2.tile(
                                [128, 512], F32, name="po", tag="aux", bufs=1
                            )
                            for g in range(4):
                                nc.tensor.matmul(
                                    po,
                                    ot_sb[t][:, g, ts(mm, 128)],
                                    wo_sb[:, g, ts(n2, 512)],
                                    start=(g == 0),
                                    stop=(g == 3),
                                )
                            ob = outp.tile(
                                [128, 512], F32, name="ob", tag="ob", bufs=3
                            )
                            nc.vector.tensor_copy(ob, po)
                            nc.sync.dma_start(
                                out=out_d[ts(m, 128), ts(n2, 512)], in_=ob
                            )

    _split_sync_waits(nc)
    return nc


def _prep_inputs(Q, K, V, w_q, w_k, w_v, w_o):
    """Build the 8 per-core input maps (host-side shard + transpose + f32r)."""
    Q = np.asarray(Q, dtype=np.float32)
    K = np.asarray(K, dtype=np.float32)
    V = np.asarray(V, dtype=np.float32)
    w_q = np.asarray(w_q, dtype=np.float32)
    w_k = np.asarray(w_k, dtype=np.float32)
    w_v = np.asarray(w_v, dtype=np.float32)
    w_o = np.asarray(w_o, dtype=np.float32)

    masks = np.zeros((4, 128, QT), dtype=np.float32)
    k_idx = np.arange(128)[:, None]
    q_idx = np.arange(QT)[None, :]
    for r in range(4):
        masks[r] = (k_idx <= q_idx - 128 * r).astype(np.float32)
    onescol = np.ones((128, 8), dtype=np.float32)
    sel = np.zeros((8, 4, 128), dtype=np.float32)
    for g in range(4):
        sel[2 * g, g, 0:64] = 1.0
        sel[2 * g + 1, g, 64:128] = 1.0

    qT = [_round_f32r(Q[b].T) for b in range(B)]
    kT = [_round_f32r(K[b].T) for b in range(B)]
    vT = [_round_f32r(V[b].T) for b in range(B)]
    wqT = [_round_f32r(w_q[hg * 512 : hg * 512 + 512, :].T) for hg in range(2)]
    wkT = [_round_f32r(w_k[hg * 512 : hg * 512 + 512, :].T) for hg in range(2)]
    wvT = [_round_f32r(w_v[hg * 512 : hg * 512 + 512, :].T) for hg in range(2)]
    woT = [_round_f32r(w_o[:, hg * 512 : hg * 512 + 512].T) for hg in range(2)]

    in_maps = []
    for c in range(N_CORES):
        b, hg = c // 2, c % 2
        in_maps.append(
            {
                "qT": qT[b],
                "kT": kT[b],
                "vT": vT[b],
                "wqT": wqT[hg],
                "wkT": wkT[hg],
                "wvT": wvT[hg],
                "woT": woT[hg],
                "masks": masks,
                "onescol": onescol,
                "sel": sel,
            }
        )
    return in_maps


def kernel(Q, K, V, w_q, w_k, w_v, w_o, _trace=False):
    from concourse.bass_utils import run_bass_kernel_spmd

    if "nc" not in _CACHE:
        _CACHE["nc"] = build_nc()
    nc = _CACHE["nc"]

    in_maps = _prep_inputs(Q, K, V, w_q, w_k, w_v, w_o)
    res = run_bass_kernel_spmd(
        nc, in_maps, core_ids=list(range(N_CORES)), trace=_trace
    )
    outs = [r["out"] for r in res.results]
    full = np.empty((B, N, D_MODEL), dtype=np.float32)
    for b in range(B):
        full[b] = outs[2 * b] + outs[2 * b + 1]
    if _trace:
        _CACHE["last_result"] = res
    return full


# revision 8
# speedup vs baseline: 1.3982x; 1.3982x over previous
"""Multi-head causal attention (b=4, n=2048, d_model=1024, 16 heads) on 8
Trainium2 NeuronCores.

Sharding: core c = (batch b = c//2, head-group hg = c%2); each core computes
one batch with 8 heads (tensor-parallel split of w_q/w_k/w_v by rows and w_o
by columns) and returns a partial [2048, 1024] output; host sums the two
head-group partials per batch.

Per-core device algorithm (all matmuls fp32r = 1 PE cycle/column):
  Phase 1: qT/kT = (X @ W.T).T via PE with host-transposed inputs; v in
           natural [seq, d] layout with an appended ones column (gives
           softmax denominators for free in the PV matmul).
  Phase 2: per q-tile t (512 q) and head-pair g: scores S^T[k,q] blocks via
           2-way row-tiled matmuls (dk=64 each), exp on ACT (scale=1/8,
           no max subtraction: |s|/8 < ~3), causal mask multiply on diagonal
           blocks, PV accumulation into [65, 512] PSUM (row 64 = rowsum).
           Stage O^T + rowsums to SBUF, reciprocal, PE-broadcast, rescale.
  Phase 3: O-projection out[seq, 1024] = O^T.T @ w_o_slice.T per q-tile.
"""

import numpy as np

B = 4
N = 2048
D_MODEL = 1024
DK = 64
NT = 4          # q tiles of 512
QT = 512        # q tile size
KB = 128        # key block size
N_CORES = 8

_CACHE = {}


def _round_f32r(x: np.ndarray) -> np.ndarray:
    """fp16 conversion for device inputs (RNE)."""
    return np.ascontiguousarray(x, dtype=np.float32).astype(np.float16)


def _split_sync_waits(nc, max_waits=1):
    """walrus on this image allows only 1 sync-wait command per instruction;
    hoist excess waits onto same-engine NoOps inserted just before."""
    import concourse.mybir as mybir

    n_split = 0
    for fn in nc.m.functions:
        for blk in fn.blocks:
            insts = list(blk.instructions)
            out = []
            for inst in insts:
                si = inst.sync_info
                if si is not None and len(si.on_wait) > max_waits:
                    waits = list(si.on_wait)
                    head, rest = waits[:-max_waits], waits[-max_waits:]
                    while head:
                        chunk, head = head[:max_waits], head[max_waits:]
                        nop = mybir.InstNoOp(
                            name=f"{inst.name}-ws{n_split}-{len(out)}",
                            engine=inst.engine,
                            opcode="NoOp",
                            sync_info=mybir.SyncInfo(on_wait=chunk, on_update=[]),
                            bass_nofuse=True,
                        )
                        out.append(nop)
                    si.on_wait = rest
                    n_split += 1
                out.append(inst)
            if len(out) != len(insts):
                blk.instructions = out
    return n_split


def build_nc():
    import concourse.bass as bass
    import concourse.mybir as mybir
    import concourse.tile as tile
    from concourse.bass import ts

    F32 = mybir.dt.float32
    F32R = mybir.dt.float16  # compute/storage dtype for all matmul operands
    AF = mybir.ActivationFunctionType

    nc = bass.Bass("TRN2", target_bir_lowering=False, debug=False)

    qT_d = nc.dram_tensor("qT", [D_MODEL, N], F32R, kind="ExternalInput")
    kT_d = nc.dram_tensor("kT", [D_MODEL, N], F32R, kind="ExternalInput")
    vT_d = nc.dram_tensor("vT", [D_MODEL, N], F32R, kind="ExternalInput")
    wqT_d = nc.dram_tensor("wqT", [D_MODEL, 512], F32R, kind="ExternalInput")
    wkT_d = nc.dram_tensor("wkT", [D_MODEL, 512], F32R, kind="ExternalInput")
    wvT_d = nc.dram_tensor("wvT", [D_MODEL, 512], F32R, kind="ExternalInput")
    woT_d = nc.dram_tensor("woT", [512, D_MODEL], F32R, kind="ExternalInput")
    masks_d = nc.dram_tensor("masks", [4, 128, QT], F32R, kind="ExternalInput")
    onescol_d = nc.dram_tensor("onescol", [128, 8], F32R, kind="ExternalInput")
    sel_d = nc.dram_tensor("sel", [8, 4, 128], F32R, kind="ExternalInput")
    out_d = nc.dram_tensor("out", [N, D_MODEL], F32, kind="ExternalOutput")

    with (
        tile.TileContext(nc) as tc,
        nc.allow_low_precision(reason="fp32r matmuls are intentional"),
    ):
        with (
            tc.tile_pool(name="persist", bufs=1) as persist,
            tc.tile_pool(name="pt_pool", bufs=1) as pt_pool,
            tc.tile_pool(name="outp", bufs=1) as outp,
        ):
            # ---- persistent SBUF tensors (whole-kernel lifetime) ----
            qT_all = persist.tile([128, 4, N], F32R)   # [part, m-block, seq]
            kT_all = persist.tile([128, 4, N], F32R)
            v_all = persist.tile([128, 16, 8, 65], F32R)  # [k-part, sb, head, d+1]
            onescol_sb = persist.tile([128, 8], F32R)
            sel_sb = persist.tile([8, 4, 128], F32R)
            nc.sync.dma_start(out=onescol_sb, in_=onescol_d[:, :])
            nc.sync.dma_start(out=sel_sb, in_=sel_d[:, :, :])

            # ================= Phase 1: projections =================
            with (
                tc.tile_pool(name="w1", bufs=1) as w1,
                tc.tile_pool(name="xs", bufs=8) as xs,
                tc.tile_pool(name="pp", bufs=1, space="PSUM") as pp,
            ):
                wq_sb = w1.tile([128, 8, 512], F32R)
                wk_sb = w1.tile([128, 8, 512], F32R)
                wv_sb = w1.tile([128, 8, 512], F32R)
                for kc in range(8):
                    nc.sync.dma_start(out=wq_sb[:, kc, :], in_=wqT_d[ts(kc, 128), :])
                    nc.sync.dma_start(out=wk_sb[:, kc, :], in_=wkT_d[ts(kc, 128), :])
                    nc.sync.dma_start(out=wv_sb[:, kc, :], in_=wvT_d[ts(kc, 128), :])

                # q/k projections: qT_all[:, m, tsl] = (W X^T) block
                for src_d, w_sb, dst in (
                    (qT_d, wq_sb, qT_all),
                    (kT_d, wk_sb, kT_all),
                ):
                    for t in range(NT):
                        pj = [
                            pp.tile(
                                [128, QT], F32, name=f"pj{m}", tag=f"pj{m}", bufs=2
                            )
                            for m in range(4)
                        ]
                        for kc in range(8):
                            x_t = xs.tile([128, QT], F32R, name="x_t", tag="x_t")
                            nc.sync.dma_start(
                                out=x_t, in_=src_d[ts(kc, 128), ts(t, QT)]
                            )
                            for m in range(4):
                                nc.tensor.matmul(
                                    pj[m],
                                    w_sb[:, kc, ts(m, 128)],
                                    x_t[:, :],
                                    start=(kc == 0),
                                    stop=(kc == 7),
                                )
                        for m in range(4):
                            nc.vector.tensor_copy(dst[:, m, ts(t, QT)], pj[m])

                # v projection: natural [seq, d] layout + ones column
                for t in range(NT):
                    pj = [
                        pp.tile([128, QT], F32, name=f"pj{m}", tag=f"pj{m}", bufs=2)
                        for m in range(4)
                    ]
                    for kc in range(8):
                        x_t = xs.tile([128, QT], F32R, name="x_t", tag="x_t")
                        nc.sync.dma_start(out=x_t, in_=vT_d[ts(kc, 128), ts(t, QT)])
                        for m in range(4):
                            nc.tensor.matmul(
                                pj[m],
                                x_t[:, ts(m, 128)],
                                wv_sb[:, kc, :],
                                start=(kc == 0),
                                stop=(kc == 7),
                            )
                    for m in range(4):
                        sb = t * 4 + m
                        nc.vector.tensor_copy(
                            v_all[:, sb, :, 0:64],
                            pj[m][:, :].rearrange("p (h d) -> p h d", h=8),
                        )
                        nc.vector.tensor_copy(v_all[:, sb, :, 64], onescol_sb)

            # ================= Phase 2+3: attention + O-projection =========
            with (
                tc.tile_pool(name="persist2", bufs=1) as persist2,
                tc.tile_pool(name="ps2", bufs=1, space="PSUM") as ps2,
            ):
                ot_sb = [
                    persist2.tile([128, 4, QT], F32R, name=f"ot_sb{t}", tag=f"ot{t}")
                    for t in range(NT)
                ]
                rs_sb = [
                    persist2.tile([8, QT], F32R, name=f"rs_sb{t}", tag=f"rs{t}")
                    for t in range(NT)
                ]
                recip_sb = [
                    persist2.tile([8, QT], F32R, name=f"recip{t}", tag=f"rc{t}")
                    for t in range(NT)
                ]
                masks_sb = persist2.tile([128, 4, QT], F32R)
                wo_sb = persist2.tile([128, 4, D_MODEL], F32R)

                for r in range(4):
                    nc.sync.dma_start(out=masks_sb[:, r, :], in_=masks_d[r, :, :])
                for g in range(4):
                    nc.sync.dma_start(out=wo_sb[:, g, :], in_=woT_d[ts(g, 128), :])

                for t in range(NT):
                    nkb = 4 * t + 4  # causal: key blocks 0 .. 4t+3 (even count)
                    npair = nkb // 2
                    for g in range(4):
                        ota = ps2.tile([65, QT], F32, name="ota", tag="ota", bufs=1)
                        otb = ps2.tile([65, QT], F32, name="otb", tag="otb", bufs=1)
                        for p in range(npair):
                            j0, j1 = 2 * p, 2 * p + 1
                            sa2 = ps2.tile(
                                [128, 2 * QT], F32, name="sa2", tag="sa2", bufs=1
                            )
                            sb2 = ps2.tile(
                                [128, 2 * QT], F32, name="sb2", tag="sb2", bufs=1
                            )
                            for half, j in ((0, j0), (1, j1)):
                                nc.tensor.matmul(
                                    sa2[:, half * QT : (half + 1) * QT],
                                    kT_all[0:64, g, ts(j, 128)],
                                    qT_all[0:64, g, ts(t, QT)],
                                    start=True,
                                    stop=True,
                                    tile_position=(0, 0),
                                )
                                nc.tensor.matmul(
                                    sb2[:, half * QT : (half + 1) * QT],
                                    kT_all[64:128, g, ts(j, 128)],
                                    qT_all[64:128, g, ts(t, QT)],
                                    start=True,
                                    stop=True,
                                    tile_position=(64, 0),
                                )
                            pta = pt_pool.tile(
                                [128, 2 * QT], F32R, name="pta", tag="pta", bufs=4
                            )
                            ptb = pt_pool.tile(
                                [128, 2 * QT], F32R, name="ptb", tag="ptb", bufs=4
                            )
                            nc.scalar.activation(pta, sa2, AF.Exp, scale=0.125)
                            nc.scalar.activation(ptb, sb2, AF.Exp, scale=0.125)
                            for half, j in ((0, j0), (1, j1)):
                                r = j - 4 * t
                                if r >= 0:
                                    sl = slice(half * QT, (half + 1) * QT)
                                    nc.vector.tensor_mul(
                                        pta[:, sl], pta[:, sl], masks_sb[:, r, :]
                                    )
                                    nc.vector.tensor_mul(
                                        ptb[:, sl], ptb[:, sl], masks_sb[:, r, :]
                                    )
                            for half, j in ((0, j0), (1, j1)):
                                sl = slice(half * QT, (half + 1) * QT)
                                nc.tensor.matmul(
                                    ota,
                                    v_all[:, j, 2 * g, :],
                                    pta[:, sl],
                                    start=(j == 0),
                                    stop=(j == nkb - 1),
                                )
                                nc.tensor.matmul(
                                    otb,
                                    v_all[:, j, 2 * g + 1, :],
                                    ptb[:, sl],
                                    start=(j == 0),
                                    stop=(j == nkb - 1),
                                )
                        # stage O^T and rowsums to SBUF
                        nc.vector.tensor_copy(ot_sb[t][0:64, g, :], ota[0:64, :])
                        nc.vector.tensor_copy(ot_sb[t][64:128, g, :], otb[0:64, :])
                        tmp_rs = pt_pool.tile(
                            [1, 2, QT], F32R, name="tmp_rs", tag="tmp_rs", bufs=2
                        )
                        nc.vector.tensor_copy(tmp_rs[0:1, 0, :], ota[64:65, :])
                        nc.vector.tensor_copy(tmp_rs[0:1, 1, :], otb[64:65, :])
                        nc.sync.dma_start(
                            out=rs_sb[t][2 * g : 2 * g + 2, :], in_=tmp_rs[0:1, :, :]
                        )

                    # normalize this q-tile: recip of gathered rowsums,
                    # PE-broadcast per head pair via K=8 selector matmul
                    nc.vector.reciprocal(recip_sb[t], rs_sb[t])
                    for g in range(4):
                        bc = ps2.tile([128, QT], F32, name="bc", tag="aux", bufs=2)
                        nc.tensor.matmul(
                            bc,
                            sel_sb[:, g, :],
                            recip_sb[t][:, :],
                            start=True,
                            stop=True,
                        )
                        nc.vector.tensor_mul(
                            ot_sb[t][:, g, :], ot_sb[t][:, g, :], bc
                        )

                    # ---- Phase 3 for this q-tile: O-projection ----
                    for mm in range(4):
                        m = 4 * t + mm
                        for n2 in range(2):
                            po = ps2.tile(
                                [128, 512], F32, name="po", tag="aux", bufs=2
                            )
                            for g in range(4):
                                nc.tensor.matmul(
                                    po,
                                    ot_sb[t][:, g, ts(mm, 128)],
                                    wo_sb[:, g, ts(n2, 512)],
                                    start=(g == 0),
                                    stop=(g == 3),
                                )
                            ob = outp.tile(
                                [128, 512], F32, name="ob", tag="ob", bufs=3
                            )
                            nc.vector.tensor_copy(ob, po)
                            nc.sync.dma_start(
                                out=out_d[ts(m, 128), ts(n2, 512)], in_=ob
                            )

    _split_sync_waits(nc)
    return nc


def _prep_inputs(Q, K, V, w_q, w_k, w_v, w_o):
    """Build the 8 per-core input maps (host-side shard + transpose + f32r)."""
    Q = np.asarray(Q, dtype=np.float32)
    K = np.asarray(K, dtype=np.float32)
    V = np.asarray(V, dtype=np.float32)
    w_q = np.asarray(w_q, dtype=np.float32)
    w_k = np.asarray(w_k, dtype=np.float32)
    w_v = np.asarray(w_v, dtype=np.float32)
    w_o = np.asarray(w_o, dtype=np.float32)

    masks = np.zeros((4, 128, QT), dtype=np.float16)
    k_idx = np.arange(128)[:, None]
    q_idx = np.arange(QT)[None, :]
    for r in range(4):
        masks[r] = (k_idx <= q_idx - 128 * r).astype(np.float16)
    onescol = np.ones((128, 8), dtype=np.float16)
    sel = np.zeros((8, 4, 128), dtype=np.float16)
    for g in range(4):
        sel[2 * g, g, 0:64] = 1.0
        sel[2 * g + 1, g, 64:128] = 1.0

    qT = [_round_f32r(Q[b].T) for b in range(B)]
    kT = [_round_f32r(K[b].T) for b in range(B)]
    vT = [_round_f32r(V[b].T) for b in range(B)]
    wqT = [_round_f32r(w_q[hg * 512 : hg * 512 + 512, :].T) for hg in range(2)]
    wkT = [_round_f32r(w_k[hg * 512 : hg * 512 + 512, :].T) for hg in range(2)]
    wvT = [_round_f32r(w_v[hg * 512 : hg * 512 + 512, :].T) for hg in range(2)]
    woT = [_round_f32r(w_o[:, hg * 512 : hg * 512 + 512].T) for hg in range(2)]

    in_maps = []
    for c in range(N_CORES):
        b, hg = c // 2, c % 2
        in_maps.append(
            {
                "qT": qT[b],
                "kT": kT[b],
                "vT": vT[b],
                "wqT": wqT[hg],
                "wkT": wkT[hg],
                "wvT": wvT[hg],
                "woT": woT[hg],
                "masks": masks,
                "onescol": onescol,
                "sel": sel,
            }
        )
    return in_maps


def kernel(Q, K, V, w_q, w_k, w_v, w_o, _trace=False):
    from concourse.bass_utils import run_bass_kernel_spmd

    if "nc" not in _CACHE:
        _CACHE["nc"] = build_nc()
    nc = _CACHE["nc"]

    in_maps = _prep_inputs(Q, K, V, w_q, w_k, w_v, w_o)
    res = run_bass_kernel_spmd(
        nc, in_maps, core_ids=list(range(N_CORES)), trace=_trace
    )
    outs = [r["out"] for r in res.results]
    full = np.empty((B, N, D_MODEL), dtype=np.float32)
    for b in range(B):
        full[b] = outs[2 * b] + outs[2 * b + 1]
    if _trace:
        _CACHE["last_result"] = res
    return full
